# revision 15
# baseline (speedup 1.0000x reference)
"""Trainium2 Bass kernel for nn_KCLWONegLoss.

Reference math (all f32):
    sums    = embs.sum(axis=1)                          # [64, 512]
    pos[p]  = cos(sums[p], sums[p+8])                   # p in 0..55
    a       = g1[neg1]; b = g2[neg2]                    # [56, 32, 512]
    sim[p,d]= cos over K axis (32) of a[p,:,d], b[p,:,d]
    num     = exp(pos/0.1)
    den     = num + sum_d exp(sim/0.1)
    loss    = 2 * sum_p (log(den) - pos/0.1)

Sharding: data-parallel over the D=64 group axis (8 groups/core) for the
embs reduction; the 56 positive pairs are sharded 7/core, each core
receiving only its 7*32 gathered rows of g1/g2 (row-gather done host-side
at shard-build time; the device still reads every gathered byte from HBM).

Device layout (every DMA is fully contiguous per partition):
  gc   [128, 24 + 4*512]: selector consts (24 cols) + the 4 gather
       half-planes (a0,a1,b0,b1), 8.1 KiB/partition, one DMA, lands first
       so the negative path finishes mid-stream.
  embs [128, 16, 512]: flat row r=16p+h of the [2048, 512] shard lives at
       partition p, slot h -> 32 KiB contiguous per partition. Streamed in
       decreasing-size h-chunks (alternating the two HWDGE rings) so the
       tail chunk is tiny and the last group-sum matmul is DMA-gated.
Group sums are 16 selector matmuls on the otherwise-idle PE array
accumulating [8,512] in PSUM; Vector only does the negative-path
element-wise products; Scalar does rsqrt/exp. The 56 cosines + log-sum
are assembled on host in float64 from the per-core [8,512]+[8] outputs.
"""

import numpy as np

D, NG, DIM = 64, 256, 512
L, K = 8, 32
P = D - L
TEMP = 0.1
EPS = 1e-8
N_CORES = 8
GPC = D // N_CORES
PPC = P // N_CORES
NH = 16
CHUNKS = [4, 4, 3, 2, 2, 1]
GOFF = 24

_PROGRAM = None
LAST_RESULTS = None


def _build_program():
    import concourse.bass as bass
    import concourse.tile as tile
    from concourse import bacc, mybir

    f32 = mybir.dt.float32
    f32r = mybir.dt.float32r
    AF = mybir.ActivationFunctionType
    nc = bacc.Bacc("TRN2", target_bir_lowering=False, debug=False)

    embs_t = nc.dram_tensor("embs_s", [128, NH, DIM], f32, kind="ExternalInput")
    gc_t = nc.dram_tensor("gc", [128, GOFF + 4 * DIM], f32, kind="ExternalInput")
    sums_a_t = nc.dram_tensor("sums_a", [GPC, DIM // 2], f32, kind="ExternalOutput")
    sums_b_t = nc.dram_tensor("sums_b", [GPC, DIM // 2], f32, kind="ExternalOutput")
    den_t = nc.dram_tensor("den_out", [8, 1], f32, kind="ExternalOutput")

    with tile.TileContext(nc) as tc:
        with (
            tc.tile_pool(name="pool", bufs=1) as pool,
            tc.tile_pool(name="psum", bufs=1, space=bass.MemorySpace.PSUM) as psum,
        ):
            gc = pool.tile([128, GOFF + 4 * DIM], f32r, tag="gc")
            nc.sync.dma_start(gc[:], gc_t.ap().bitcast(f32r))

            echunks = []
            h0 = 0
            for ci, hn in enumerate(CHUNKS):
                e = pool.tile([128, hn, DIM], f32r, tag=f"e{ci}")
                eng = nc.sync if ci % 2 == 0 else nc.scalar
                eng.dma_start(
                    e[:], embs_t.ap()[:, h0:h0 + hn, :].bitcast(f32r)
                )
                echunks.append((e, h0, hn))
                h0 += hn

            sel, blk0, blk1 = gc[:, 0:8], gc[:, 8:16], gc[:, 16:24]

            sq = pool.tile([128, 4 * DIM], f32r, tag="sq")
            nc.vector.tensor_mul(sq[:], gc[:, GOFF:], gc[:, GOFF:])
            prod = pool.tile([128, 2 * DIM], f32r, tag="prod")
            nc.vector.tensor_mul(prod[:], gc[:, GOFF:GOFF + 2 * DIM],
                                 gc[:, GOFF + 2 * DIM:])

            dot_ps = psum.tile([8, DIM], f32, tag="dot")
            asq_ps = psum.tile([8, DIM], f32, tag="asq")
            bsq_ps = psum.tile([8, DIM], f32, tag="bsq")
            nc.tensor.matmul(asq_ps[:], blk0, sq[:, 0:DIM], start=True, stop=False)
            nc.tensor.matmul(bsq_ps[:], blk0, sq[:, 2 * DIM:3 * DIM], start=True, stop=False)
            nc.tensor.matmul(asq_ps[:], blk1, sq[:, DIM:2 * DIM], start=False, stop=True)
            nc.tensor.matmul(bsq_ps[:], blk1, sq[:, 3 * DIM:], start=False, stop=True)
            nc.tensor.matmul(dot_ps[:], blk0, prod[:, 0:DIM], start=True, stop=False)
            nc.tensor.matmul(dot_ps[:], blk1, prod[:, DIM:], start=False, stop=True)

            ai = pool.tile([8, DIM], f32, tag="ai")
            bi = pool.tile([8, DIM], f32, tag="bi")
            nc.scalar.activation(ai[:], asq_ps[:], AF.Abs_reciprocal_sqrt)
            nc.scalar.activation(bi[:], bsq_ps[:], AF.Abs_reciprocal_sqrt)
            rr = pool.tile([8, DIM], f32, tag="rr")
            nc.vector.tensor_mul(rr[:], ai[:], bi[:])
            sim = pool.tile([8, DIM], f32, tag="sim")
            nc.vector.tensor_mul(sim[:], dot_ps[:], rr[:])
            ex = pool.tile([8, DIM], f32, tag="ex")
            den = pool.tile([8, 1], f32, tag="den")
            nc.scalar.activation(
                ex[:], sim[:], AF.Exp,
                scale=float(1.0 / TEMP), accum_out=den[:],
            )
            nc.sync.dma_start(den_t.ap(), den[:])

            sums_ps = psum.tile([GPC, DIM], f32, tag="sums")
            for (e, h0, hn) in echunks:
                for j in range(hn):
                    h = h0 + j
                    nc.tensor.matmul(
                        sums_ps[:], sel, e[:, j, :],
                        start=(h == 0), stop=(h == NH - 1),
                    )
            # tail: ONE copy (two engines reading the same PSUM tile
            # serialize anyway), then ship the two column-halves on the two
            # HWDGE rings concurrently
            sums_sb = pool.tile([GPC, DIM], f32, tag="sums_sb")
            nc.scalar.copy(sums_sb[:], sums_ps[:])
            nc.sync.dma_start(sums_a_t.ap(), sums_sb[:, 0:DIM // 2])
            nc.scalar.dma_start(sums_b_t.ap(), sums_sb[:, DIM // 2:])

    nc.compile()
    return nc


def _get_program():
    global _PROGRAM
    if _PROGRAM is None:
        _PROGRAM = _build_program()
    return _PROGRAM


def kernel(embs, g0, g1, g2, neg1, neg2, **_unused):
    global LAST_RESULTS
    from concourse.bass_utils import run_bass_kernel_spmd

    embs = np.ascontiguousarray(np.asarray(embs, dtype=np.float32))
    g1 = np.ascontiguousarray(np.asarray(g1, dtype=np.float32))
    g2 = np.ascontiguousarray(np.asarray(g2, dtype=np.float32))
    neg1 = np.asarray(neg1).astype(np.int64)
    neg2 = np.asarray(neg2).astype(np.int64)

    consts = np.zeros((128, GOFF), np.float32)
    for g in range(GPC):
        consts[16 * g:16 * g + 16, g] = 1.0
    for m in range(4):
        consts[32 * m:32 * m + 32, 8 + m] = 1.0
        consts[32 * m:32 * m + 32, 20 + m] = 1.0

    in_maps = []
    for c in range(N_CORES):
        idx1 = neg1[c * PPC:(c + 1) * PPC].reshape(-1)
        idx2 = neg2[c * PPC:(c + 1) * PPC].reshape(-1)
        gab = np.ones((128, 4, DIM), np.float32)
        gab[:, 0, :] = g1[idx1[0:128]]
        gab[0:96, 1, :] = g1[idx1[128:224]]
        gab[:, 2, :] = g2[idx2[0:128]]
        gab[0:96, 3, :] = g2[idx2[128:224]]
        gc = np.empty((128, GOFF + 4 * DIM), np.float32)
        gc[:, :GOFF] = consts
        gc[:, GOFF:] = gab.reshape(128, 4 * DIM)
        in_maps.append({
            "embs_s": embs[c * GPC:(c + 1) * GPC].reshape(128, NH, DIM),
            "gc": gc,
        })

    nc = _get_program()
    res = run_bass_kernel_spmd(nc, in_maps, core_ids=list(range(N_CORES)))
    LAST_RESULTS = res

    sums = np.concatenate(
        [np.concatenate([res.results[c]["sums_a"], res.results[c]["sums_b"]],
                        axis=1) for c in range(N_CORES)], axis=0
    ).astype(np.float64)
    den_neg = np.concatenate(
        [res.results[c]["den_out"][:PPC, 0] for c in range(N_CORES)]
    ).astype(np.float64)

    s_i, s_j = sums[:P], sums[L:]
    na = np.maximum(np.sqrt((s_i * s_i).sum(1)), EPS)
    nb = np.maximum(np.sqrt((s_j * s_j).sum(1)), EPS)
    pos = (s_i * s_j).sum(1) / (na * nb)
    num = np.exp(pos / TEMP)
    den = num + den_neg
    total = 2.0 * np.sum(np.log(den) - pos / TEMP)
    return np.asarray(total, dtype=np.float32)
